# revision 34
# baseline (speedup 1.0000x reference)
"""BitMGQA forward for Trainium2, 8-core SPMD Bass/Tile kernel.

Sharding: the B*S = 4096 tokens are split into 8 slices of 512 (cores 0-3 =
batch 0, cores 4-7 = batch 1). Each core quantizes + projects its own token
slice for Q/K/V; the per-batch K/V (plus their per-token output scales) are
AllGathered across each 4-core batch group in one merged collective;
attention, layernorm and the output projection are then fully local to each
core's 512 query tokens.

BitLinear structure is exploited: activation quantization produces integers in
[-128, 127] and weight binarization produces +-1, both exactly representable
in bf16, so every projection matmul runs on the PE in bf16 with fp32 PSUM
accumulation exactly; rmsnorm / quant / weight scales fold into one per-token
fp32 scale applied to the matmul output (or, for K, into the softmax exp's
per-key scale operand).
"""
import contextlib

import numpy as np

import concourse.bass as bass
import concourse.mybir as mybir
import concourse.tile as tile

dt = mybir.dt
AF = mybir.ActivationFunctionType
ALU = mybir.AluOpType
AX = mybir.AxisListType

# problem dims (hardcoded per contract)
B, S, D = 2, 2048, 1024
N_CORES = 8
GRP = 4                    # cores per batch group
TPC = (B * S) // N_CORES   # 512 tokens per core
P = 128
NTC = TPC // P             # 4 token tiles per core
DC = D // P                # 8 contraction chunks
KVE = 256
QH, KVH, HD = 8, 2, 128
NKC = S // P               # 16 key chunks per batch
MAGIC = 12582912.0         # 1.5 * 2**23: (x + MAGIC) - MAGIC == rint(x)
INV_SQRT_HD = float(HD) ** -0.5
LN_EPS = 1e-5
RMS_EPS = 1e-6

_BUILT = {}


def _split_multiwaits(nc, max_waits=1):
    """The pinned walrus rejects >1 sync-wait per instruction ("Too many sync
    wait commands"). Split extras onto single-wait NoOps inserted before the
    offending instruction on the same engine (sequencer stalls in order, so
    semantics are identical)."""
    for f in nc.m.functions:
        for bb in f.blocks:
            insts = list(bb.instructions)
            if not any(
                i.sync_info is not None and len(i.sync_info.on_wait) > max_waits
                for i in insts
            ):
                continue
            new_insts = []
            for ins in insts:
                si = ins.sync_info
                if si is not None and len(si.on_wait) > max_waits:
                    waits = list(si.on_wait)
                    for k, w in enumerate(waits[:-max_waits]):
                        new_insts.append(mybir.InstNoOp(
                            name=f"{ins.name}-wsplit{k}",
                            engine=ins.engine,
                            sync_info=mybir.SyncInfo(on_wait=[w], on_update=[]),
                        ))
                    ins.sync_info = mybir.SyncInfo(
                        on_wait=waits[-max_waits:], on_update=list(si.on_update)
                    )
                new_insts.append(ins)
            bb.instructions = new_insts


class _Emit:
    """Per-build emission state."""

    def __init__(self, nc, tc_, ctx):
        self.nc = nc
        self.tc = tc_
        f32 = dt.float32
        self.small = ctx.enter_context(tc_.tile_pool(name="small", bufs=1))
        self.persist = ctx.enter_context(tc_.tile_pool(name="persist", bufs=1))
        self.pipe = ctx.enter_context(tc_.tile_pool(name="pipe", bufs=3))
        self.live = ctx.enter_context(tc_.tile_pool(name="live", bufs=1))
        self.dram = ctx.enter_context(
            tc_.tile_pool(name="dram", bufs=1, space="DRAM"))
        self.ones = self.small.tile([P, P], f32, tag="ones128", name="ones")
        nc.vector.memset(self.ones[:], 1.0)
        self.eps_rms = self.small.tile([P, 1], f32, tag="eps_rms", name="eps_rms")
        nc.vector.memset(self.eps_rms[:], float(D * RMS_EPS))
        self.eps_ln = self.small.tile([P, 1], f32, tag="eps_ln", name="eps_ln")
        nc.vector.memset(self.eps_ln[:], LN_EPS)

    # ---- helpers -------------------------------------------------------
    def weight_prep(self, w_dram, n_oc, swT, name, psred, wpool=None,
                    load_eng=None):
        """sign(w)^T into swT [P, n_oc, DC, P] bf16; returns mean|w| as a
        [P, 1] fp32 column replicated across partitions. Loads weight rows
        two 128-row chunks at a time to halve DMA count."""
        nc = self.nc
        wpool = wpool or self.wpool
        npair = n_oc // 2
        abscol = self.small.tile(
            [P, npair], dt.float32, tag=f"abscol_{name}", name=f"abscol_{name}")
        wv = w_dram.rearrange("(oc p) d -> p oc d", p=P)
        for pr in range(npair):
            wt = wpool.tile([P, 2, D], dt.float32, tag="wtile", bufs=2,
                            name="wt")
            (load_eng or nc.sync).dma_start(wt[:], wv[:, 2 * pr:2 * pr + 2])
            sw = wpool.tile([P, 2, D], dt.bfloat16, tag="swtile", bufs=2,
                            name="sw")
            nc.scalar.sign(sw[:], wt[:])
            nc.vector.tensor_reduce(
                abscol[:, pr:pr + 1], wt[:], AX.XY, ALU.add,
                apply_absolute_value=True,
            )
            for i in range(2):
                nc.sync.dma_start_transpose(swT[:, 2 * pr + i], sw[:, i])
        rowtot = self.small.tile(
            [P, 1], dt.float32, tag=f"rowtot_{name}", name=f"rowtot_{name}")
        nc.vector.reduce_sum(rowtot[:], abscol[:], axis=AX.X)
        ps = psred.tile([P, 1], dt.float32, tag="psred", bufs=2, name="psred")
        nc.tensor.matmul(ps[:], self.ones[:], rowtot[:], start=True, stop=True)
        wm = self.small.tile([P, 1], dt.float32, tag=f"wm_{name}", name=f"wm_{name}")
        nc.scalar.mul(wm[:], ps[:], 1.0 / (n_oc * P * D))
        return wm

    def load_x(self, x_dram, tagpfx):
        """Load a [TPC, D] activation as two [P, 2, D] tiles; return the four
        [P, D] per-token-tile APs (token t = tc*128 + p)."""
        xv = x_dram.rearrange("(h tc p) d -> p h tc d", p=P, h=2)
        aps = []
        for h in range(2):
            xt = self.live.tile([P, 2, D], dt.float32, tag=f"xt2_{h}",
                                name=f"{tagpfx}{h}")
            self.nc.sync.dma_start(xt[:], xv[:, h])
            aps.extend([xt[:, 0, :], xt[:, 1, :]])
        return aps

    def quant(self, specs):
        """specs: list of (x_aps, GT, name). Quantize several tensors with
        ACT ops grouped by function (fewer activation-table switches).
        Returns dict name -> os [P, NTC] raw out-scale."""
        nc = self.nc
        sm = self.small

        def st(tag, name):
            return sm.tile([P, NTC], dt.float32, tag=f"{tag}_{name}",
                           name=f"{tag}_{name}")

        scr = self.pipe.tile([P, D], dt.float32, tag="scr", bufs=1, name="scr")
        stats = {}
        for x_aps, GT, name in specs:
            ssq, amax = st("ssq", name), st("amax", name)
            for tc in range(NTC):
                nc.scalar.activation(
                    scr[:], x_aps[tc], AF.Square, accum_out=ssq[:, tc:tc + 1])
                nc.vector.tensor_reduce(
                    amax[:, tc:tc + 1], x_aps[tc], AX.X, ALU.max,
                    apply_absolute_value=True,
                )
            stats[name] = (ssq, amax)
        out = {}
        for x_aps, GT, name in specs:
            ssq, amax = stats[name]
            u = st("u", name)
            nc.scalar.activation(u[:], ssq[:], AF.Sqrt, bias=self.eps_rms[:])
            c, amn, os, ra, m1 = (st("c", name), st("amn", name),
                                  st("os", name), st("ra", name), st("m1", name))
            nc.vector.reciprocal(c[:], u[:])
            nc.vector.tensor_tensor(amn[:], c[:], amax[:], ALU.mult)
            nc.vector.tensor_scalar_max(amn[:], amn[:], 1e-5)
            nc.vector.tensor_scalar_mul(os[:], amn[:], 1.0 / 127.0)
            nc.vector.reciprocal(ra[:], amn[:])
            nc.vector.tensor_tensor(m1[:], c[:], ra[:], ALU.mult)
            nc.vector.tensor_scalar_mul(m1[:], m1[:], 127.0)
            out[name] = (os, m1)
        for x_aps, GT, name in specs:
            os, m1 = out[name]
            for tc in range(NTC):
                tr = self.pipe.tile([P, D], dt.float32, tag="tr", bufs=2,
                                    name="tr")
                nc.scalar.activation(
                    tr[:], x_aps[tc], AF.Copy, bias=MAGIC,
                    scale=m1[:, tc:tc + 1])
                g = self.pipe.tile([P, D], dt.bfloat16, tag="gtile", bufs=3,
                                   name="g")
                nc.vector.tensor_scalar_sub(g[:], tr[:], MAGIC)
                nc.sync.dma_start_transpose(GT[:, tc], g[:])
        return {name: out[name][0] for _, _, name in specs}

    def os_row(self, os_col, name):
        """[P, NTC] per-token column (t = tc*128+p) -> [P, TPC] fp32
        broadcast row via a DRAM bounce."""
        nc = self.nc
        scratch = self.dram.tile([1, TPC], dt.float32, tag=f"osrow_d_{name}",
                                 name=f"osrow_d_{name}")
        nc.gpsimd.dma_start(scratch[0].rearrange("(c p) -> p c", p=P), os_col[:])
        row = self.small.tile([P, TPC], dt.float32, tag=f"osrow_{name}",
                              name=f"osrow_{name}")
        nc.gpsimd.dma_start(row[:], scratch[:].to_broadcast((P, TPC)))
        return row

    def mul_wm(self, os, wm, name, extra=None):
        out = self.small.tile([P, NTC], dt.float32, tag=f"oss_{name}",
                              name=f"oss_{name}")
        self.nc.vector.tensor_tensor(
            out[:], os[:], wm[:, 0:1].to_broadcast((P, NTC)), ALU.mult)
        if extra is not None:
            self.nc.vector.tensor_scalar_mul(out[:], out[:], extra)
        return out


def build_nc(zb: bool, zln: bool):
    """zb: all projection biases zero; zln: ln_g == 1 and ln_b == 0."""
    nc = bass.Bass()
    f32, bf16 = dt.float32, dt.bfloat16

    xq_d = nc.dram_tensor("xq", [TPC, D], f32, kind="ExternalInput")
    xk_d = nc.dram_tensor("xk", [TPC, D], f32, kind="ExternalInput")
    xv_d = nc.dram_tensor("xv", [TPC, D], f32, kind="ExternalInput")
    wq_d = nc.dram_tensor("wq", [D, D], f32, kind="ExternalInput")
    wk_d = nc.dram_tensor("wk", [KVE, D], f32, kind="ExternalInput")
    wv_d = nc.dram_tensor("wv", [KVE, D], f32, kind="ExternalInput")
    wo_d = nc.dram_tensor("wo", [D, D], f32, kind="ExternalInput")
    if not zb:
        bq_d = nc.dram_tensor("bq", [1, D], f32, kind="ExternalInput")
        bk_d = nc.dram_tensor("bk", [1, KVE], f32, kind="ExternalInput")
        bv_d = nc.dram_tensor("bv", [1, KVE], f32, kind="ExternalInput")
        bo_d = nc.dram_tensor("bo", [1, D], f32, kind="ExternalInput")
    if not zln:
        g_d = nc.dram_tensor("g_ln", [1, D], f32, kind="ExternalInput")
        bl_d = nc.dram_tensor("b_ln", [1, D], f32, kind="ExternalInput")
    y_d = nc.dram_tensor("y", [TPC, D], f32, kind="ExternalOutput")

    groups = [[0, 1, 2, 3], [4, 5, 6, 7]]

    with tile.TileContext(nc) as tc_, contextlib.ExitStack() as ctx:
        em = _Emit(nc, tc_, ctx)
        small, persist, pipe, dram = em.small, em.persist, em.pipe, em.dram

        # persistent SBUF structures
        swqT = persist.tile([P, QH, DC, P], bf16, tag="swqT", name="swqT")
        swoT = persist.tile([P, QH, DC, P], bf16, tag="swoT", name="swoT")
        swkT = persist.tile([P, KVH, DC, P], bf16, tag="swkT", name="swkT")
        swvT = persist.tile([P, KVH, DC, P], bf16, tag="swvT", name="swvT")
        GlnT = persist.tile([P, NTC, DC, P], bf16, tag="GlnT", name="GlnT")
        qT_sb = persist.tile([P, NTC, DC, P], bf16, tag="qT_sb", name="qT_sb")
        kT_sb = persist.tile([P, KVH, S], bf16, tag="kT_sb", name="kT_sb")
        v_aug = [persist.tile([P, NKC, 132], bf16, tag=f"vaug{kv}",
                              name=f"vaug{kv}") for kv in range(KVH)]
        x_sb = persist.tile([P, NTC, D], f32, tag="x_sb", name="x_sb")
        # os2_all[:, 0, :] = key scales, [:, 1, :] = value scales
        os2_all = persist.tile([P, 2, NKC], f32, tag="os2_all", name="os2_all")
        # incremental layernorm stats, accumulated per attention head-group
        s1p = persist.tile([P, NTC, 4], f32, tag="s1p", name="s1p")
        ssqp = persist.tile([P, NTC, 4], f32, tag="ssqp", name="ssqp")

        mid_cm = tc_.tile_pool(name="mid", bufs=1)
        mid = mid_cm.__enter__()
        em.wpool = mid
        GqT = mid.tile([P, NTC, DC, P], bf16, tag="GqT", name="GqT")
        qtok = mid.tile([P, NTC, D], bf16, tag="qtok", name="qtok")
        GkT = mid.tile([P, NTC, DC, P], bf16, tag="GkT", name="GkT")
        GvT = mid.tile([P, NTC, DC, P], bf16, tag="GvT", name="GvT")

        # merged collective buffer:
        #   [ k ints (oc p t) | v ints (tc p o) | osk (p c) f32 | osv (p c) f32 ]
        CC_K, CC_V = 0, KVE * TPC
        CC_SC = CC_V + TPC * KVE
        CC_N = CC_SC + 4 * TPC  # two [P, NTC] f32 scale tiles as bf16 slots
        cc_in = dram.tile([CC_N], bf16, tag="cc_in", name="cc_in")
        cc_out = dram.tile([GRP, CC_N], bf16, tag="cc_out", name="cc_out")

        with tc_.tile_pool(name="ps1", bufs=1, space="PSUM") as ps1:
            # ---------- quantize local k/v slices ----------
            xk_t = em.load_x(xk_d, "xk")
            xv_t = em.load_x(xv_d, "xv")

            # ---------- kv weights (loads ride the ACT hwdge queue) ----------
            wmk = em.weight_prep(wk_d, KVH, swkT, "k", ps1)
            wmv = em.weight_prep(wv_d, KVH, swvT, "v", ps1)

            oss = em.quant([(xk_t, GkT, "k"), (xv_t, GvT, "v")])
            osk_s = em.mul_wm(oss["k"], wmk, "k")
            osv_s = em.mul_wm(oss["v"], wmv, "v")

            # scale scatters go first on the gpsimd queue (ready earliest)
            sc_f32 = cc_in[CC_SC:CC_N].bitcast(f32)
            if zb:
                nc.gpsimd.dma_start(
                    sc_f32[0:TPC].rearrange("(p c) -> p c", p=P), osk_s[:])
                nc.gpsimd.dma_start(
                    sc_f32[TPC:2 * TPC].rearrange("(p c) -> p c", p=P), osv_s[:])
            else:
                onecol = small.tile([P, NTC], f32, tag="onecol", name="onecol")
                nc.vector.memset(onecol[:], 1.0)
                nc.gpsimd.dma_start(
                    sc_f32[0:TPC].rearrange("(p c) -> p c", p=P), onecol[:])
                nc.gpsimd.dma_start(
                    sc_f32[TPC:2 * TPC].rearrange("(p c) -> p c", p=P), onecol[:])

            # ---------- local K/V projections (raw ints in bf16) ----------
            kT_loc = mid.tile([P, KVH, TPC], bf16, tag="kT_loc", name="kT_loc")
            v_loc = mid.tile([P, NTC, KVE], bf16, tag="v_loc", name="v_loc")
            if not zb:
                bk_sb = small.tile([P, KVH], f32, tag="bk_sb", name="bk_sb")
                nc.sync.dma_start(bk_sb[:], bk_d[0].rearrange("(c p) -> p c", p=P))
                oskb_row = em.os_row(osk_s, "oskb")
                vb_row = small.tile([P, KVE], f32, tag="vb_row", name="vb_row")
                nc.gpsimd.dma_start(vb_row[:], bv_d[:].to_broadcast((P, KVE)))
            # K: [o, t] orientation
            for oc in range(KVH):
                psum = ps1.tile([P, TPC], f32, tag="proj", bufs=2, name="pj")
                for dc_ in range(DC):
                    nc.tensor.matmul(
                        psum[:], swkT[:, oc, dc_, :], GkT[:, :, dc_, :],
                        start=(dc_ == 0), stop=(dc_ == DC - 1),
                    )
                if zb:
                    nc.vector.tensor_copy(kT_loc[:, oc], psum[:])
                else:
                    tmp = pipe.tile([P, TPC], f32, tag="kvtmp", bufs=2,
                                    name="kvtmp")
                    nc.vector.tensor_tensor(tmp[:], psum[:], oskb_row[:], ALU.mult)
                    nc.vector.tensor_scalar(
                        kT_loc[:, oc], tmp[:], bk_sb[:, oc:oc + 1], None, ALU.add)
            nc.gpsimd.dma_start(
                cc_in[CC_K:CC_V].rearrange("(oc p t) -> p oc t", p=P, t=TPC),
                kT_loc[:])
            # V: token-major [t, o] orientation, straight into the cc layout
            for tcc in range(NTC):
                psum = ps1.tile([P, KVE], f32, tag="projv", bufs=2, name="pv")
                for dc_ in range(DC):
                    nc.tensor.matmul(
                        psum[:], GvT[:, tcc, dc_, :],
                        swvT[:, :, dc_, :],
                        start=(dc_ == 0), stop=(dc_ == DC - 1),
                    )
                if zb:
                    nc.vector.tensor_copy(v_loc[:, tcc], psum[:])
                else:
                    tmp2 = pipe.tile([P, KVE], f32, tag="vtmp", bufs=2,
                                     name="vtmp")
                    nc.vector.tensor_scalar(
                        tmp2[:], psum[:], osv_s[:, tcc:tcc + 1], None, ALU.mult)
                    nc.vector.tensor_tensor(
                        v_loc[:, tcc], tmp2[:], vb_row[:], ALU.add)
            nc.gpsimd.dma_start(
                cc_in[CC_V:CC_SC].rearrange("(tc p o) -> p tc o", p=P, o=KVE),
                v_loc[:])

            # ---------- single merged collective ----------
            nc.gpsimd.collective_compute(
                "AllGather", ALU.bypass, replica_groups=groups,
                ins=[cc_in.opt()], outs=[cc_out.opt()])

            # ---------- overlaps collective: q/o weights + Q ----------
            xq_t = em.load_x(xq_d, "xq")
            wmq = em.weight_prep(wq_d, QH, swqT, "q", ps1)
            osq = em.quant([(xq_t, GqT, "q")])["q"]
            osq_s = em.mul_wm(osq, wmq, "q", extra=INV_SQRT_HD)
            if not zb:
                qb_row = small.tile([P, D], f32, tag="qb_row", name="qb_row")
                nc.gpsimd.dma_start(qb_row[:], bq_d[:].to_broadcast((P, D)))
                # reference scales q (incl. bias) by 1/sqrt(hd)
                nc.vector.tensor_scalar_mul(qb_row[:], qb_row[:], INV_SQRT_HD)
            # Q token-major: per-partition scale, then dma-transpose to d-major
            for tcc in range(NTC):
                for og in range(2):
                    psum = ps1.tile([P, TPC], f32, tag="proj", bufs=2, name="pj")
                    for dc_ in range(DC):
                        nc.tensor.matmul(
                            psum[:], GqT[:, tcc, dc_, :],
                            swqT[:, og * 4:(og + 1) * 4, dc_, :],
                            start=(dc_ == 0), stop=(dc_ == DC - 1),
                        )
                    if zb:
                        nc.vector.tensor_scalar(
                            qtok2[:, og, 0:TPC] if False else qtok[:, tcc, og * TPC:(og + 1) * TPC],
                            psum[:], osq_s[:, tcc:tcc + 1], None, ALU.mult)
                    else:
                        tmp = pipe.tile([P, TPC], f32, tag="qtmp", bufs=2,
                                        name="qtmp")
                        nc.vector.tensor_scalar(
                            tmp[:], psum[:], osq_s[:, tcc:tcc + 1], None, ALU.mult)
                        nc.vector.tensor_tensor(
                            qtok[:, tcc, og * TPC:(og + 1) * TPC], tmp[:],
                            qb_row[:, og * TPC:(og + 1) * TPC], ALU.add)
            for tcc in range(NTC):
                nc.sync.dma_start_transpose(qT_sb[:, tcc], qtok[:, tcc, :])

            # ---------- land gathered K/V (one DMA per source core) ----------
            vraw = mid.tile([P, NKC, KVE], bf16, tag="vraw", name="vraw")
            for s_ in range(GRP):
                nc.sync.dma_start(
                    kT_sb[:, :, s_ * TPC:(s_ + 1) * TPC],
                    cc_out[s_, CC_K:CC_V].rearrange("(oc p t) -> p oc t",
                                                    p=P, t=TPC),
                )
                nc.sync.dma_start(
                    vraw[:, s_ * NTC:(s_ + 1) * NTC, :],
                    cc_out[s_, CC_V:CC_SC].rearrange("(tc p o) -> p tc o",
                                                     p=P, o=KVE),
                )
                nc.sync.dma_start(
                    os2_all[:, :, s_ * NTC:(s_ + 1) * NTC],
                    cc_out[s_, CC_SC:CC_N].bitcast(f32)
                    .rearrange("(r p c) -> p r c", p=P, r=2),
                )
            for kv in range(KVH):
                nc.vector.memset(v_aug[kv][:, :, 128:129], 1.0)
                nc.vector.tensor_tensor(
                    v_aug[kv][:, :, 0:P],
                    vraw[:, :, kv * P:(kv + 1) * P],
                    os2_all[:, 1, :, None].to_broadcast((P, NKC, P)),
                    ALU.mult,
                )

        mid_cm.__exit__(None, None, None)

        # ---------- attention ----------
        with (
            tc_.tile_pool(name="ps2", bufs=1, space="PSUM") as ps2,
            tc_.tile_pool(name="probsp", bufs=1) as probsp,
        ):
            for kv in range(KVH):
                for hp in range(2):
                    probs = probsp.tile(
                        [P, NKC, 2, TPC], dt.bfloat16, tag="probs", bufs=2,
                        name="probs")
                    for kc in range(NKC):
                        ps_s = ps2.tile([P, 2 * TPC], dt.float32, tag="scores",
                                        bufs=2, name="ps_s")
                        for j in range(2):
                            h = kv * 4 + hp * 2 + j
                            nc.tensor.matmul(
                                ps_s[:, j * TPC:(j + 1) * TPC],
                                kT_sb[:, kv, kc * P:(kc + 1) * P],
                                qT_sb[:, :, h, :],
                                start=True, stop=True,
                            )
                        nc.scalar.activation(
                            probs[:, kc],
                            ps_s[:].rearrange("p (j t) -> p j t", j=2),
                            AF.Exp, scale=os2_all[:, 0, kc:kc + 1],
                        )
                    for j in range(2):
                        h = kv * 4 + hp * 2 + j
                        for tcc in range(NTC):
                            ps_o = ps2.tile([P, 132], dt.float32, tag="av",
                                            bufs=4, name="ps_o")
                            for kc in range(NKC):
                                nc.tensor.matmul(
                                    ps_o[:, 0:129],
                                    probs[:, kc, j, tcc * P:(tcc + 1) * P],
                                    v_aug[kv][:, kc, 0:129],
                                    start=(kc == 0), stop=(kc == NKC - 1),
                                )
                            rden = small.tile([P, 1], dt.float32, tag="rden",
                                              bufs=4, name="rden")
                            nc.vector.reciprocal(rden[:], ps_o[:, 128:129])
                            nc.vector.tensor_scalar(
                                x_sb[:, tcc, h * P:(h + 1) * P],
                                ps_o[:, 0:P], rden[:], None, ALU.mult,
                            )
                    grp = kv * 2 + hp
                    h0 = kv * 4 + hp * 2
                    scrA = pipe.tile([P, 2 * P], dt.float32, tag="scrA",
                                     bufs=1, name="scrA")
                    for tcc in range(NTC):
                        xg = x_sb[:, tcc, h0 * P:(h0 + 2) * P]
                        nc.vector.reduce_sum(
                            s1p[:, tcc, grp:grp + 1], xg, axis=AX.X)
                        nc.vector.tensor_tensor(scrA[:], xg, xg, ALU.mult)
                        nc.vector.reduce_sum(
                            ssqp[:, tcc, grp:grp + 1], scrA[:], axis=AX.X)

        # ---------- layernorm + final quant + output projection ----------
        with (
            tc_.tile_pool(name="ps3", bufs=1, space="PSUM") as ps3,
            tc_.tile_pool(name="opool", bufs=1) as opool,
        ):
            wmo = em.weight_prep(wo_d, QH, swoT, "o", ps3, wpool=opool)
            sm = small

            def st(tag):
                return sm.tile([P, NTC], dt.float32, tag=tag, name=tag)

            s1, ssql = st("s1_ln"), st("ssq_ln")
            nc.vector.reduce_sum(s1[:], s1p[:], axis=AX.X)
            nc.vector.reduce_sum(ssql[:], ssqp[:], axis=AX.X)
            mu, e2, m2, var, sd, rstd, nmu = (
                st("mu"), st("e2"), st("m2"), st("var"), st("sd"), st("rstd"),
                st("nmu"))
            ssq2, amax2 = st("ssq2"), st("amax2")
            u2, c2, amn2, osl, ra2, m1l = (
                st("u2"), st("c2"), st("amn2"), st("osl"), st("ra2"), st("m1l"))
            syl = st("syl")
            scr2 = pipe.tile([P, D], dt.float32, tag="scr", bufs=1, name="scr2")

            if not zln:
                g_row = persist.tile([P, D], dt.float32, tag="g_row", name="g_row")
                nc.gpsimd.dma_start(g_row[:], g_d[:].to_broadcast((P, D)))
                b_row = persist.tile([P, D], dt.float32, tag="b_row", name="b_row")
                nc.gpsimd.dma_start(b_row[:], bl_d[:].to_broadcast((P, D)))
            if not zb:
                ob_row = persist.tile([P, D], dt.float32, tag="ob_row",
                                      name="ob_row")
                nc.gpsimd.dma_start(ob_row[:], bo_d[:].to_broadcast((P, D)))

            yv = y_d.rearrange("(tc p) o -> p tc o", p=P)
            # two-token-tile halves: out-proj of half 0 overlaps the ln/quant
            # chain of half 1
            for hf in range(2):
                hs = slice(2 * hf, 2 * hf + 2)
                nc.vector.tensor_scalar_mul(mu[:, hs], s1[:, hs], 1.0 / D)
                nc.vector.tensor_scalar_mul(e2[:, hs], ssql[:, hs], 1.0 / D)
                nc.vector.tensor_tensor(m2[:, hs], mu[:, hs], mu[:, hs], ALU.mult)
                nc.vector.tensor_tensor(var[:, hs], e2[:, hs], m2[:, hs],
                                        ALU.subtract)
                nc.scalar.activation(sd[:, hs], var[:, hs], AF.Sqrt,
                                     bias=em.eps_ln[:])
                nc.vector.reciprocal(rstd[:, hs], sd[:, hs])
                nc.vector.tensor_scalar_mul(nmu[:, hs], mu[:, hs], -1.0)

                lt2 = em.live.tile([P, 2, D], dt.float32, tag=f"xt2_{hf}",
                                   name=f"lt2_{hf}")
                lt_aps = []
                for i in range(2):
                    tcc = 2 * hf + i
                    nc.vector.tensor_scalar(
                        lt2[:, i, :], x_sb[:, tcc], nmu[:, tcc:tcc + 1],
                        rstd[:, tcc:tcc + 1], ALU.add, ALU.mult,
                    )
                    if not zln:
                        nc.vector.tensor_tensor(
                            lt2[:, i, :], lt2[:, i, :], g_row[:], ALU.mult)
                        nc.vector.tensor_tensor(
                            lt2[:, i, :], lt2[:, i, :], b_row[:], ALU.add)
                    lt_aps.append(lt2[:, i, :])

                # bitlinear quant of this half
                for i, lt in enumerate(lt_aps):
                    tcc = 2 * hf + i
                    nc.scalar.activation(
                        scr2[:], lt, AF.Square, accum_out=ssq2[:, tcc:tcc + 1])
                    nc.vector.tensor_reduce(
                        amax2[:, tcc:tcc + 1], lt, AX.X, ALU.max,
                        apply_absolute_value=True)
                nc.scalar.activation(u2[:, hs], ssq2[:, hs], AF.Sqrt,
                                     bias=em.eps_rms[:])
                nc.vector.reciprocal(c2[:, hs], u2[:, hs])
                nc.vector.tensor_tensor(amn2[:, hs], c2[:, hs], amax2[:, hs],
                                        ALU.mult)
                nc.vector.tensor_scalar_max(amn2[:, hs], amn2[:, hs], 1e-5)
                nc.vector.tensor_scalar_mul(osl[:, hs], amn2[:, hs], 1.0 / 127.0)
                nc.vector.reciprocal(ra2[:, hs], amn2[:, hs])
                nc.vector.tensor_tensor(m1l[:, hs], c2[:, hs], ra2[:, hs],
                                        ALU.mult)
                nc.vector.tensor_scalar_mul(m1l[:, hs], m1l[:, hs], 127.0)
                nc.vector.tensor_tensor(
                    syl[:, hs], osl[:, hs],
                    wmo[:, 0:1].to_broadcast((P, 2)), ALU.mult)
                for i, lt in enumerate(lt_aps):
                    tcc = 2 * hf + i
                    tr = pipe.tile([P, D], dt.float32, tag="tr", bufs=2,
                                   name="tr")
                    nc.scalar.activation(
                        tr[:], lt, AF.Copy, bias=MAGIC,
                        scale=m1l[:, tcc:tcc + 1])
                    g = pipe.tile([P, D], dt.bfloat16, tag="gtile", bufs=3,
                                  name="g")
                    nc.vector.tensor_scalar_sub(g[:], tr[:], MAGIC)
                    nc.sync.dma_start_transpose(GlnT[:, tcc], g[:])
                for i in range(2):
                    tcc = 2 * hf + i
                    for og in range(2):
                        psum = ps3.tile([P, TPC], dt.float32, tag="yproj",
                                        bufs=3, name="py")
                        for dc_ in range(DC):
                            nc.tensor.matmul(
                                psum[:], GlnT[:, tcc, dc_, :],
                                swoT[:, og * 4:(og + 1) * 4, dc_, :],
                                start=(dc_ == 0), stop=(dc_ == DC - 1),
                            )
                        yt = pipe.tile([P, TPC], dt.float32, tag="yt", bufs=3,
                                       name="yt")
                        nc.vector.tensor_scalar(
                            yt[:], psum[:], syl[:, tcc:tcc + 1], None, ALU.mult)
                        if not zb:
                            nc.vector.tensor_tensor(
                                yt[:], yt[:],
                                ob_row[:, og * TPC:(og + 1) * TPC], ALU.add)
                        nc.sync.dma_start(
                            yv[:, tcc, og * TPC:(og + 1) * TPC], yt[:])

    _split_multiwaits(nc)
    return nc


def kernel(**inputs):
    from concourse.bass_utils import run_bass_kernel_spmd

    def arr(name):
        return np.ascontiguousarray(np.asarray(inputs[name], dtype=np.float32))

    q, k, v = arr("query"), arr("key"), arr("value")
    qw, kw, vw, ow = arr("q_w"), arr("k_w"), arr("v_w"), arr("out_w")
    qb, kb, vb, ob = arr("q_b"), arr("k_b"), arr("v_b"), arr("out_b")
    lg, lb = arr("ln_g"), arr("ln_b")

    zb = not (qb.any() or kb.any() or vb.any() or ob.any())
    zln = bool(np.all(lg == 1.0)) and not lb.any()

    key = (zb, zln)
    if key not in _BUILT:
        _BUILT[key] = build_nc(zb, zln)
    nc = _BUILT[key]

    qf = q.reshape(B * S, D)
    kf = k.reshape(B * S, D)
    vf = v.reshape(B * S, D)
    in_maps = []
    for c in range(N_CORES):
        sl = slice(c * TPC, (c + 1) * TPC)
        m = {
            "xq": qf[sl], "xk": kf[sl], "xv": vf[sl],
            "wq": qw, "wk": kw, "wv": vw, "wo": ow,
        }
        if not zb:
            m["bq"] = qb.reshape(1, D)
            m["bk"] = kb.reshape(1, KVE)
            m["bv"] = vb.reshape(1, KVE)
            m["bo"] = ob.reshape(1, D)
        if not zln:
            m["g_ln"] = lg.reshape(1, D)
            m["b_ln"] = lb.reshape(1, D)
        in_maps.append(m)

    res = run_bass_kernel_spmd(nc, in_maps, core_ids=list(range(N_CORES)))
    y = np.concatenate([res.results[c]["y"] for c in range(N_CORES)], axis=0)
    return y.reshape(B, S, D).astype(np.float32)
